# revision 1
# baseline (speedup 1.0000x reference)
"""Trainium2 Bass kernel for nn_CopyMechanism.

Math (per batch b):
  out[g,c] = softmax_c(mask ? (score_h[g]+score_c[c]) : -inf)
             * sigmoid(gate_h[g]+gate_c[c]+b0)

The softmax over c of (score_h[g] + score_c[c]) equals softmax_c(score_c)
because score_h[g] is constant along c — copy_probs is independent of g and
w_attn[:H] drops out entirely. encoder_output is unused by the reference.
Scores are O(1) (unit-normal ctx, tiny weights), so exp needs no max
subtraction — softmax output is identical up to rounding.

Per core (1 batch of 8):
  sc[c] = ctx[c,:] @ wa_c   and   gc[c] = ctx[c,:] @ wg_c
      via PE: transpose ctx 128x128 blocks into PSUM (burst), stage to SBUF
      (copies split across scalar/vector engines), then matmul with the
      [h,2] weight pair stationary, accumulating over h blocks ->
      dots land as rows [2, c] (sc row 0, gc row 1).
  gh[g] = hid[g,:] @ wg_h + b_gate   (vector mult+reduce, column layout)
  p[c]  = e[c] / Z;  e = mask ? exp(sc) : 0   (exp via sigmoid ratio:
      e^x = sig(x)/sig(-x), exactly 0 when masked);  Z via a K=32 matmul
      partition-sum, 1/Z folded into p on a [32,128] layout.
  out[g,c] = p[c] * sigmoid(gh[g] + gc[c])
      gc / p broadcast across partitions on GPSIMD (idle otherwise),
      sigmoid with per-partition bias gh on the scalar engine, final
      multiply split vector/gpsimd, direct DMA out.
"""
import sys

if "/opt/trn_rl_repo" not in sys.path:
    sys.path.insert(0, "/opt/trn_rl_repo")

import numpy as np
from contextlib import ExitStack

B, G, C, H = 8, 512, 4096, 1024
N_CORES = 8
P = 128
NCT = C // P          # 32 c-tiles of 128
NGT = G // P          # 4 g-tiles of 128
CJ = C // 512         # 8 c-chunks of 512
JH = H // P           # 8 h-blocks of 128

_cache = {}


def _build():
    import concourse.bass as bass
    import concourse.tile as tile
    from concourse import bacc, mybir
    from concourse.masks import make_identity

    f32 = mybir.dt.float32
    i32 = mybir.dt.int32
    ts = bass.ts

    nc = bacc.Bacc("TRN2", target_bir_lowering=False, debug=False,
                   num_devices=N_CORES)
    hid = nc.dram_tensor("hid", [G, H], f32, kind="ExternalInput").ap()
    ctx_d = nc.dram_tensor("ctx", [C, H], f32, kind="ExternalInput").ap()
    mask_d = nc.dram_tensor("mask", [NCT, P], i32, kind="ExternalInput").ap()
    w_d = nc.dram_tensor("w", [3, H], f32, kind="ExternalInput").ap()  # wa_c, wg_c, wg_h
    bg_d = nc.dram_tensor("bg", [1, 1], f32, kind="ExternalInput").ap()
    out_d = nc.dram_tensor("out", [G, C], f32, kind="ExternalOutput").ap()

    with tile.TileContext(nc) as tc:
        with ExitStack() as ctx:
            singles = ctx.enter_context(tc.tile_pool(name="singles", bufs=1))
            hidp = ctx.enter_context(tc.tile_pool(name="hidp", bufs=1))
            ctxp = ctx.enter_context(tc.tile_pool(name="ctxp", bufs=3))
            ctp = ctx.enter_context(tc.tile_pool(name="ctp", bufs=3))
            junkp = ctx.enter_context(tc.tile_pool(name="junkp", bufs=2))
            smp = ctx.enter_context(tc.tile_pool(name="smp", bufs=1))
            gcbp = ctx.enter_context(tc.tile_pool(name="gcbp", bufs=8))
            pbp = ctx.enter_context(tc.tile_pool(name="pbp", bufs=2))
            rowp = ctx.enter_context(tc.tile_pool(name="rowp", bufs=2))
            outp = ctx.enter_context(tc.tile_pool(name="outp", bufs=8))
            # PSUM: tp 2x2 banks + dots 2 + z 2 = 8
            tp_ps = ctx.enter_context(
                tc.tile_pool(name="tp_ps", bufs=2, space="PSUM"))
            dt_ps = ctx.enter_context(
                tc.tile_pool(name="dt_ps", bufs=2, space="PSUM"))
            z_ps_p = ctx.enter_context(
                tc.tile_pool(name="z_ps_p", bufs=2, space="PSUM"))

            # ---- tiny input DMAs first (weights feed chunk-0 dots) ----
            wpair = singles.tile([2, H], f32)
            nc.gpsimd.dma_start(out=wpair, in_=w_d[0:2, :])
            maskR = smp.tile([NCT, P], i32)
            nc.gpsimd.dma_start(out=maskR, in_=mask_d)

            # ---- ctx chunk DMAs next: transposes are the critical path.
            # Two sub-DMAs per 2MB chunk so transposes start at half-chunk.
            ctx4s = []

            def emit_ctx_dma(j, nsub=1):
                ctx4 = ctxp.tile([P, 4, H], f32, tag="ctx4")
                w = 4 // nsub
                for h2 in range(nsub):
                    nc.sync.dma_start(
                        out=ctx4[:, h2 * w:(h2 + 1) * w, :],
                        in_=ctx_d[j * 512 + h2 * w * P:
                                  j * 512 + (h2 + 1) * w * P, :].rearrange(
                            "(i p) h -> p i h", p=P))
                ctx4s.append(ctx4)

            emit_ctx_dma(0, nsub=2)
            emit_ctx_dma(1, nsub=2)
            hid4 = hidp.tile([P, NGT, H], f32)
            nc.sync.dma_start(out=hid4,
                              in_=hid.rearrange("(gi p) h -> p gi h", p=P))
            for j in range(2, CJ):
                emit_ctx_dma(j)

            # ---- constants ----
            ident = singles.tile([P, P], f32)
            make_identity(nc, ident)
            whb = singles.tile([P, H], f32)  # wg_h broadcast to all partitions
            w_gh = w_d[2:3, :]
            nc.gpsimd.dma_start(
                out=whb,
                in_=bass.AP(tensor=w_gh.tensor, offset=w_gh.offset,
                            ap=[[0, P], [1, H]]))
            bg_b = singles.tile([P, 1], f32)
            nc.gpsimd.dma_start(
                out=bg_b,
                in_=bass.AP(tensor=bg_d.tensor, offset=bg_d.offset,
                            ap=[[0, P], [1, 1]]))
            wacb = singles.tile([P, H], f32)
            w_ac = w_d[1:2, :]
            nc.gpsimd.dma_start(
                out=wacb,
                in_=bass.AP(tensor=w_ac.tensor, offset=w_ac.offset,
                            ap=[[0, P], [1, H]]))
            wgcb = singles.tile([P, H], f32)
            w_gc = w_d[0:1, :]
            nc.gpsimd.dma_start(
                out=wgcb,
                in_=bass.AP(tensor=w_gc.tensor, offset=w_gc.offset,
                            ap=[[0, P], [1, H]]))
            ones_col = singles.tile([1, P], f32)
            nc.vector.memset(ones_col, 1.0)
            ones32c = singles.tile([32, 1], f32)
            nc.vector.memset(ones32c, 1.0)

            # w2[h, 2*jh + s] = w[s, jh*128 + h] for s in {0: wg_c, 1: wa_c}
            # (gc lands on PSUM partition 0 so GPSIMD can broadcast it directly)
            w2_ps = z_ps_p.tile([P, 2 * JH], f32, tag="zps")
            for jh in range(JH):
                nc.tensor.transpose(w2_ps[:, jh * 2:jh * 2 + 2],
                                    wpair[:, ts(jh, P)], ident[0:2, 0:2])
            w2 = singles.tile([P, 2 * JH], f32)
            nc.scalar.copy(w2, w2_ps)

            # ---- gh = hid @ wg_h + b_gate  (column layout [128, NGT]) ----
            ghp = smp.tile([P, NGT], f32)
            for gi in range(NGT):
                junk = junkp.tile([P, H], f32, tag="junk")
                nc.vector.tensor_mul(junk, hid4[:, gi, :], whb)
                nc.vector.reduce_sum(ghp[:, gi:gi + 1], junk,
                                     axis=mybir.AxisListType.X)
            gh = smp.tile([P, NGT], f32)
            nc.vector.tensor_scalar(out=gh, in0=ghp, scalar1=bg_b[:, 0:1],
                                    scalar2=None, op0=mybir.AluOpType.add)

            # ---- sc, gc via PE: rows scgc[2, C] (gc row 0, sc row 1) ----
            scgc = smp.tile([2, C], f32)
            gc_bs = []
            DVE_CHUNKS = (2, 5)
            for j in range(CJ):
                ctx4 = ctx4s[j]
                if j in DVE_CHUNKS:
                    # vector-engine dot path: mult + free-dim reduce per
                    # c-tile (columns), then tiny PE transposes to rows
                    scc = smp.tile([P, 4], f32, tag=f"scc{j}")
                    gcc = smp.tile([P, 4], f32, tag=f"gcc{j}")
                    for i in range(4):
                        junk = junkp.tile([P, H], f32, tag="junk")
                        nc.vector.tensor_mul(junk, ctx4[:, i, :], wacb)
                        nc.vector.reduce_sum(scc[:, i:i + 1], junk,
                                             axis=mybir.AxisListType.X)
                        junk = junkp.tile([P, H], f32, tag="junk")
                        nc.vector.tensor_mul(junk, ctx4[:, i, :], wgcb)
                        nc.vector.reduce_sum(gcc[:, i:i + 1], junk,
                                             axis=mybir.AxisListType.X)
                    sct_ps = z_ps_p.tile([4, P], f32, tag="zps")
                    nc.tensor.transpose(sct_ps, scc, ident)
                    sct = rowp.tile([4, P], f32, tag="sct")
                    nc.scalar.copy(sct, sct_ps)
                    nc.sync.dma_start(
                        out=scgc[1:2, ts(j, 512)].rearrange(
                            "o (i p) -> o i p", p=P),
                        in_=sct)
                    gct_ps = z_ps_p.tile([4, P], f32, tag="zps")
                    nc.tensor.transpose(gct_ps, gcc, ident)
                    gct = rowp.tile([4, P], f32, tag="gct")
                    nc.scalar.copy(gct, gct_ps)
                    nc.sync.dma_start(
                        out=scgc[0:1, ts(j, 512)].rearrange(
                            "o (i p) -> o i p", p=P),
                        in_=gct)
                    gc_b = gcbp.tile([P, 512], f32, tag="gc_b")
                    nc.gpsimd.partition_broadcast(
                        gc_b, scgc[0:1, ts(j, 512)])
                    gc_bs.append(gc_b)
                    continue
                dots = dt_ps.tile([2, 512], f32, tag="dots")
                ctxTs = []
                # burst all 32 transposes (4 per h-block, 2 h-blocks per
                # PSUM tile) before the dependent dot matmuls
                for jg in range(JH // 2):
                    tp = tp_ps.tile([P, 2 * P * 4], f32, tag="tps")
                    for half in range(2):
                        jh = jg * 2 + half
                        for i in range(4):
                            nc.tensor.transpose(
                                tp[:, half * 512 + i * P:
                                   half * 512 + (i + 1) * P],
                                ctx4[:, i, ts(jh, P)], ident)
                    ctxT = ctp.tile([P, 2 * P * 4], f32, tag="ctxT")
                    nc.scalar.copy(ctxT, tp)
                    ctxTs.append(ctxT)
                for jg in range(JH // 2):
                    for half in range(2):
                        jh = jg * 2 + half
                        nc.tensor.matmul(
                            dots, w2[:, jh * 2:jh * 2 + 2],
                            ctxTs[jg][:, half * 512:(half + 1) * 512],
                            start=(jh == 0), stop=(jh == JH - 1))
                nc.scalar.copy(scgc[:, ts(j, 512)], dots)
                gc_b = gcbp.tile([P, 512], f32, tag="gc_b")
                nc.gpsimd.partition_broadcast(gc_b, scgc[0:1, ts(j, 512)])
                gc_bs.append(gc_b)

            # ---- masked softmax over c (on [NCT, 128] layout), no max
            # subtraction (scores are O(1)) ----
            sc2 = smp.tile([NCT, P], f32)
            nc.gpsimd.dma_start(
                out=sc2,
                in_=scgc[1:2, :].rearrange("o (ci p) -> o ci p", p=P))
            msc = smp.tile([NCT, P], f32)
            nc.vector.memset(msc, -1e30)
            nc.vector.copy_predicated(msc, maskR, sc2)
            # e^x = sigmoid(x) / sigmoid(-x); exactly 0 for masked entries
            s1 = smp.tile([NCT, P], f32)
            nc.scalar.activation(s1, msc, mybir.ActivationFunctionType.Sigmoid)
            s2 = smp.tile([NCT, P], f32)
            nc.scalar.activation(s2, msc, mybir.ActivationFunctionType.Sigmoid,
                                 scale=-1.0)
            r2 = smp.tile([NCT, P], f32)
            nc.vector.reciprocal(r2, s2)
            e = smp.tile([NCT, P], f32)
            nc.vector.tensor_mul(e, s1, r2)
            z_col = smp.tile([NCT, 1], f32)
            nc.vector.reduce_sum(z_col, e, axis=mybir.AxisListType.X)
            z_ps = z_ps_p.tile([1, 1], f32, tag="zps")
            nc.tensor.matmul(z_ps, z_col, ones32c, start=True, stop=True)
            z_sb = smp.tile([1, 1], f32)
            nc.scalar.copy(z_sb, z_ps)
            rz = smp.tile([1, 1], f32)
            nc.vector.reciprocal(rz, z_sb)
            zc_ps = z_ps_p.tile([NCT, 1], f32, tag="zps")
            nc.tensor.matmul(zc_ps, ones_col[0:1, 0:NCT], rz,
                             start=True, stop=True)
            rz_col = smp.tile([NCT, 1], f32)
            nc.scalar.copy(rz_col, zc_ps)
            pT = smp.tile([NCT, P], f32)
            nc.vector.tensor_scalar(out=pT, in0=e, scalar1=rz_col[:, 0:1],
                                    scalar2=None, op0=mybir.AluOpType.mult)
            p_row = smp.tile([1, C], f32)
            nc.gpsimd.dma_start(
                out=p_row[0:1, :].rearrange("o (ci p) -> o ci p", p=P),
                in_=pT)

            # ---- output: out[g,c] = sigmoid(gh[g] + gc[c]) * p[c] ----
            for j in range(CJ):
                p_b = pbp.tile([P, 512], f32, tag="p_b")
                nc.gpsimd.partition_broadcast(p_b, p_row[0:1, ts(j, 512)])
                for gi in range(NGT):
                    out_t = outp.tile([P, 512], f32, tag="out_t")
                    nc.scalar.activation(
                        out_t, gc_bs[j],
                        mybir.ActivationFunctionType.Sigmoid,
                        bias=gh[:, gi:gi + 1])
                    nc.vector.tensor_mul(out_t, out_t, p_b)
                    nc.sync.dma_start(
                        out=out_d[ts(gi, P), ts(j, 512)], in_=out_t)

    nc.compile()
    return nc


def _get_nc():
    if "nc" not in _cache:
        _cache["nc"] = _build()
    return _cache["nc"]


def make_w3(w_attn, w_gate):
    # rows: (wg_c, wa_c, wg_h) — gc weight first so gc lands on partition 0
    return np.ascontiguousarray(
        np.stack([w_gate[H:], w_attn[H:], w_gate[:H]], axis=0),
        dtype=np.float32)


def kernel(hidden_states, context_hidden, encoder_output, w_attn, w_gate,
           b_gate, copy_mask):
    from concourse.bass_utils import run_bass_kernel_spmd

    nc = _get_nc()
    w3 = make_w3(w_attn, w_gate)
    bg = np.asarray(b_gate, dtype=np.float32).reshape(1, 1)
    in_maps = []
    for b in range(B):
        in_maps.append({
            "hid": np.ascontiguousarray(hidden_states[b], dtype=np.float32),
            "ctx": np.ascontiguousarray(context_hidden[b], dtype=np.float32),
            "mask": np.ascontiguousarray(
                copy_mask[b].reshape(NCT, P).astype(np.int32)),
            "w": w3,
            "bg": bg,
        })
    res = run_bass_kernel_spmd(nc, in_maps, core_ids=list(range(N_CORES)))
    return np.stack([res.results[b]["out"] for b in range(B)], axis=0)



# revision 6
# speedup vs baseline: 1.4712x; 1.4712x over previous
"""Trainium2 Bass kernel for nn_CopyMechanism (optimized, v3).

Math (per batch b):
  out[g,c] = softmax_c(mask ? (score_h[g]+score_c[c]) : -inf)
             * sigmoid(gate_h[g]+gate_c[c]+b0)

softmax_c(score_h[g]+score_c[c]) == softmax_c(score_c): score_h drops out,
so copy_probs is independent of g and w_attn[:H] is unused; encoder_output
is unused by the reference. Scores are O(1): no max subtraction needed.

Layout strategy: everything is pre-blocked on the host so no on-chip
transposes are needed and all HBM traffic is fp16/bf16 (13MB/core):
  - ctx arrives as [h_p=128, jh=8, c=4096] fp16; PE matmuls with the ctx
    128x128 block *stationary* (fp16 -> FWL fast weight loads) and the
    (wg_c, wa_c) column pair *moving* put sc/gc on c-partitions directly.
  - softmax + all scalings are per-partition scalar ops; gate tiles are
    [c_p=128, g=512]: ACT sigmoid(ghb + gc bias), gh broadcast once.
  - out accumulates as bf16 [128, ci, g]; host unblocks to [G, C] f32.
Pipelining: ctx streams in decreasing-size c-chunks on two DMA queues
(hid on the second queue first); per chunk: PE dots -> DVE gc copy ->
ACT gates + Exp(masked sc) -> DVE gate*e (no Z dependency). Tail is only:
tiny Z reduce, one per-chunk *1/Z tensor_scalar, and the out-DMA stream.
ACT is the pacer in the window (32*0.71us gates); DMA in+out ~37us total.
"""
import sys

if "/opt/trn_rl_repo" not in sys.path:
    sys.path.insert(0, "/opt/trn_rl_repo")

import numpy as np
from contextlib import ExitStack

B, G, C, H = 8, 512, 4096, 1024
N_CORES = 8
P = 128
JH = H // P            # 8 h-blocks of 128
NCT = C // P           # 32 c-tiles of 128
# ctx chunk sizes in c-tiles; even chunks stream on the sync HWDGE queue,
# odd chunks (+hid) on the gpsimd SWDGE queue; the last chunk is one tile
# so the only unoverlapped gate work is minimal
SZ = [5, 4, 5, 4, 5, 3, 5, 1]
NCH = len(SZ)
CB = [0]
for s in SZ:
    CB.append(CB[-1] + s)
assert CB[-1] == NCT

_cache = {}


def _build():
    import concourse.bass as bass
    import concourse.tile as tile
    from concourse import bacc, mybir

    f32 = mybir.dt.float32
    f16 = mybir.dt.float16
    bf16 = mybir.dt.bfloat16
    i32 = mybir.dt.int32
    ADD = mybir.AluOpType.add
    MULT = mybir.AluOpType.mult

    nc = bacc.Bacc("TRN2", target_bir_lowering=False, debug=False,
                   num_devices=N_CORES)
    # ctxb[p, jh, c] = ctx[c, jh*128+p]  (fp16)
    ctx_d = nc.dram_tensor("ctx", [P, JH, C], f16, kind="ExternalInput").ap()
    # hidb[p, jh, g] = hid[g, jh*128+p]  (fp16)
    hid_d = nc.dram_tensor("hid", [P, JH, G], f16, kind="ExternalInput").ap()
    # wcols[p, jh*3+s]: s=0 wg_c, s=1 wa_c, s=2 wg_h  at h=jh*128+p  (fp16)
    w_d = nc.dram_tensor("w", [P, 3 * JH], f16, kind="ExternalInput").ap()
    # maskc[p, ci] = copy_mask[ci*128+p]
    mask_d = nc.dram_tensor("mask", [P, NCT], i32, kind="ExternalInput").ap()
    bg_d = nc.dram_tensor("bg", [1, 1], f32, kind="ExternalInput").ap()
    # outb[p, ci, g] = out[g, ci*128+p]  (bf16)
    out_d = nc.dram_tensor("out", [P, NCT, G], bf16,
                           kind="ExternalOutput").ap()

    with tile.TileContext(nc) as tc:
        with ExitStack() as ctx:
            sg = ctx.enter_context(tc.tile_pool(name="sg", bufs=1))
            ps = ctx.enter_context(
                tc.tile_pool(name="ps", bufs=1, space="PSUM"))

            # ---- small input DMAs (gpsimd queue) ----
            wc = sg.tile([P, 3 * JH], f16)
            nc.gpsimd.dma_start(out=wc, in_=w_d)
            maskc = sg.tile([P, NCT], i32)
            nc.gpsimd.dma_start(out=maskc, in_=mask_d)
            bg_sb = sg.tile([1, 1], f32)
            nc.gpsimd.dma_start(out=bg_sb, in_=bg_d)
            bg_b = sg.tile([P, 1], f32)
            nc.gpsimd.partition_broadcast(bg_b, bg_sb)

            # ---- big input DMAs: even ctx chunks on the sync HWDGE
            # queue; hid + odd chunks on gpsimd (SWDGE) in parallel ----
            def ctx_chunk_dma(k):
                c0, c1 = CB[k] * P, CB[k + 1] * P
                eng = nc.sync if k % 2 == 0 else nc.gpsimd
                eng.dma_start(out=ctx_sb[:, :, c0:c1],
                              in_=ctx_d[:, :, c0:c1])

            hid_sb = sg.tile([P, JH, G], f16)
            ctx_sb = sg.tile([P, JH, C], f16)
            ctx_chunk_dma(0)
            nc.gpsimd.dma_start(out=hid_sb, in_=hid_d)
            ctx_chunk_dma(2)
            ctx_chunk_dma(1)

            ones128 = sg.tile([P, 1], f32)
            nc.vector.memset(ones128, 1.0)
            dots = ps.tile([P, 2 * NCT], f32, tag="dots")
            gc_cols = sg.tile([P, NCT], f32)
            e_cols = sg.tile([P, NCT], f32)
            msc = sg.tile([P, NCT], f32)
            nc.vector.memset(msc, -30.0)
            out_sb = sg.tile([P, NCT, G], bf16)

            def chunk_dots(k):
                for ci in range(CB[k], CB[k + 1]):
                    for jh in range(JH):
                        nc.tensor.matmul(
                            dots[:, 2 * ci:2 * ci + 2],
                            ctx_sb[:, jh, ci * P:(ci + 1) * P],
                            wc[:, jh * 3:jh * 3 + 2],
                            start=(jh == 0), stop=(jh == JH - 1))

            def chunk_post(k):
                t0, t1 = CB[k], CB[k + 1]
                # rhs col 0 = wg_c -> even dot cols are gc, odd are sc;
                # fold the gate bias b0 in here (per-partition add)
                nc.vector.tensor_scalar(
                    out=gc_cols[:, t0:t1], in0=dots[:, 2 * t0:2 * t1:2],
                    scalar1=bg_b[:, 0:1], scalar2=None, op0=ADD)
                for ci in range(t0, t1):
                    nc.scalar.activation(
                        out_sb[:, ci, :], ghb,
                        mybir.ActivationFunctionType.Sigmoid,
                        bias=gc_cols[:, ci:ci + 1])
                nc.vector.copy_predicated(
                    msc[:, t0:t1], maskc[:, t0:t1],
                    dots[:, 2 * t0 + 1:2 * t1:2])
                nc.scalar.activation(e_cols[:, t0:t1], msc[:, t0:t1],
                                     mybir.ActivationFunctionType.Exp)
                for ci in range(t0, t1):
                    nc.vector.tensor_scalar(
                        out=out_sb[:, ci, :], in0=out_sb[:, ci, :],
                        scalar1=e_cols[:, ci:ci + 1], scalar2=None,
                        op0=MULT)

            # ---- gh[g] = hid[g,:] @ wg_h -> broadcast to [128, G] ----
            # (PE does chunk-0 dots first: ctx0 and hid land in parallel)
            chunk_dots(0)
            ghp = ps.tile([1, G], f32, tag="ghp")
            for jh in range(JH):
                nc.tensor.matmul(ghp, wc[:, jh * 3 + 2:jh * 3 + 3],
                                 hid_sb[:, jh, :],
                                 start=(jh == 0), stop=(jh == JH - 1))
            gh_row = sg.tile([1, G], f32)
            nc.scalar.copy(gh_row, ghp)
            ghb = sg.tile([P, G], f32)
            nc.gpsimd.partition_broadcast(ghb, gh_row)
            for k in range(3, NCH):
                ctx_chunk_dma(k)

            chunk_post(0)
            for k in range(1, NCH):
                chunk_dots(k)
                chunk_post(k)

            # ---- Z = sum_c e; rz = 1/Z broadcast ----
            zred = sg.tile([P, 1], f32)
            nc.vector.reduce_sum(zred, e_cols, axis=mybir.AxisListType.X)
            zp = ps.tile([1, 1], f32, tag="zp")
            nc.tensor.matmul(zp, zred, ones128, start=True, stop=True)
            z_sb = sg.tile([1, 1], f32)
            nc.scalar.copy(z_sb, zp)
            rz = sg.tile([1, 1], f32)
            nc.vector.reciprocal(rz, z_sb)
            rz_b = sg.tile([P, 1], f32)
            nc.gpsimd.partition_broadcast(rz_b, rz)

            # ---- finals: one *rz per chunk (even on DVE, odd on ACT),
            # out-DMA stream split across sync + ACT HWDGE rings ----
            for k in range(NCH):
                t0, t1 = CB[k], CB[k + 1]
                if k % 2 == 0:
                    nc.vector.tensor_scalar(
                        out=out_sb[:, t0:t1, :], in0=out_sb[:, t0:t1, :],
                        scalar1=rz_b[:, 0:1], scalar2=None, op0=MULT)
                    nc.sync.dma_start(out=out_d[:, t0:t1, :],
                                      in_=out_sb[:, t0:t1, :])
                else:
                    nc.scalar.mul(out_sb[:, t0:t1, :], out_sb[:, t0:t1, :],
                                  rz_b[:, 0:1])
                    nc.scalar.dma_start(out=out_d[:, t0:t1, :],
                                        in_=out_sb[:, t0:t1, :])

    nc.compile()
    return nc


def _get_nc():
    if "nc" not in _cache:
        _cache["nc"] = _build()
    return _cache["nc"]


def make_in_maps(hidden_states, context_hidden, w_attn, w_gate, b_gate,
                 copy_mask):
    # wcols[p, jh*3+s]: s=0 wg_c, s=1 wa_c, s=2 wg_h at h=jh*128+p
    w3 = np.stack([w_gate[H:], w_attn[H:], w_gate[:H]], axis=1)  # [H, 3]
    wcols = np.ascontiguousarray(
        w3.reshape(JH, P, 3).transpose(1, 0, 2).reshape(P, 3 * JH)
    ).astype(np.float16)
    bg = np.asarray(b_gate, dtype=np.float32).reshape(1, 1)
    in_maps = []
    for b in range(B):
        ctxT = context_hidden[b].T.astype(np.float16)  # [H, C]
        ctxb = np.ascontiguousarray(
            ctxT.reshape(JH, P, C).transpose(1, 0, 2))
        hidT = hidden_states[b].T.astype(np.float16)  # [H, G]
        hidb = np.ascontiguousarray(
            hidT.reshape(JH, P, G).transpose(1, 0, 2))
        maskc = np.ascontiguousarray(
            copy_mask[b].reshape(NCT, P).T.astype(np.int32))
        in_maps.append({
            "ctx": ctxb, "hid": hidb, "w": wcols, "mask": maskc, "bg": bg,
        })
    return in_maps


def unpack_out(res):
    outs = []
    for b in range(B):
        outb = np.asarray(res.results[b]["out"])  # [P, NCT, G] bf16
        outs.append(
            outb.transpose(2, 1, 0).reshape(G, C).astype(np.float32))
    return np.stack(outs, axis=0)


def kernel(hidden_states, context_hidden, encoder_output, w_attn, w_gate,
           b_gate, copy_mask):
    from concourse.bass_utils import run_bass_kernel_spmd

    nc = _get_nc()
    in_maps = make_in_maps(hidden_states, context_hidden, w_attn, w_gate,
                           b_gate, copy_mask)
    res = run_bass_kernel_spmd(nc, in_maps, core_ids=list(range(N_CORES)))
    return unpack_out(res)


# revision 8
# speedup vs baseline: 2.0731x; 1.4091x over previous
"""Trainium2 Bass kernel for nn_CopyMechanism (optimized, v3).

Math (per batch b):
  out[g,c] = softmax_c(mask ? (score_h[g]+score_c[c]) : -inf)
             * sigmoid(gate_h[g]+gate_c[c]+b0)

softmax_c(score_h[g]+score_c[c]) == softmax_c(score_c): score_h drops out,
so copy_probs is independent of g and w_attn[:H] is unused; encoder_output
is unused by the reference. Scores are O(1): no max subtraction needed.

Layout strategy: everything is pre-blocked on the host so no on-chip
transposes are needed and all HBM traffic is fp16/bf16 (13MB/core):
  - ctx arrives as [h_p=128, jh=8, c=4096] fp16; PE matmuls with the ctx
    128x128 block *stationary* (fp16 -> FWL fast weight loads) and the
    (wg_c, wa_c) column pair *moving* put sc/gc on c-partitions directly.
  - softmax + all scalings are per-partition scalar ops; gate tiles are
    [c_p=128, g=512]: ACT sigmoid(ghb + gc bias), gh broadcast once.
  - out accumulates as bf16 [128, ci, g]; host unblocks to [G, C] f32.
Pipelining: ctx streams in decreasing-size c-chunks on two DMA queues
(hid on the second queue first); per chunk: PE dots -> DVE gc copy ->
ACT gates + Exp(masked sc) -> DVE gate*e (no Z dependency). Tail is only:
tiny Z reduce, one per-chunk *1/Z tensor_scalar, and the out-DMA stream.
ACT is the pacer in the window (32*0.71us gates); DMA in+out ~37us total.
"""
import sys

if "/opt/trn_rl_repo" not in sys.path:
    sys.path.insert(0, "/opt/trn_rl_repo")

import numpy as np
from contextlib import ExitStack

B, G, C, H = 8, 512, 4096, 1024
N_CORES = 8
P = 128
JH = H // P            # 8 h-blocks of 128
NCT = C // P           # 32 c-tiles of 128
# ctx chunk sizes in c-tiles; even chunks stream on the sync HWDGE queue,
# odd chunks (+hid) on the gpsimd SWDGE queue; the last chunk is one tile
# so the only unoverlapped gate work is minimal
SZ = [5, 4, 5, 4, 5, 3, 5, 1]
NCH = len(SZ)
CB = [0]
for s in SZ:
    CB.append(CB[-1] + s)
assert CB[-1] == NCT

_cache = {}


def _build():
    import concourse.bass as bass
    import concourse.tile as tile
    from concourse import bacc, mybir

    f32 = mybir.dt.float32
    f16 = mybir.dt.float16
    bf16 = mybir.dt.bfloat16
    i32 = mybir.dt.int32
    ADD = mybir.AluOpType.add
    MULT = mybir.AluOpType.mult

    nc = bacc.Bacc("TRN2", target_bir_lowering=False, debug=False,
                   num_devices=N_CORES)
    # ctxb[p, jh, c] = ctx[c, jh*128+p]  (fp16)
    ctx_d = nc.dram_tensor("ctx", [P, JH, C], f16, kind="ExternalInput").ap()
    # hidb[p, jh, g] = hid[g, jh*128+p]  (fp16)
    hid_d = nc.dram_tensor("hid", [P, JH, G], f16, kind="ExternalInput").ap()
    # wcols[p, jh*3+s]: s=0 wg_c, s=1 wa_c, s=2 wg_h  at h=jh*128+p  (fp16)
    w_d = nc.dram_tensor("w", [P, 3 * JH], f16, kind="ExternalInput").ap()
    # maskc[p, ci] = copy_mask[ci*128+p]
    mask_d = nc.dram_tensor("mask", [P, NCT], i32, kind="ExternalInput").ap()
    bg_d = nc.dram_tensor("bg", [1, 1], f32, kind="ExternalInput").ap()
    # outb[p, ci, g] = out[g, ci*128+p]  (bf16)
    out_d = nc.dram_tensor("out", [P, NCT, G], bf16,
                           kind="ExternalOutput").ap()

    with tile.TileContext(nc) as tc:
        with ExitStack() as ctx:
            sg = ctx.enter_context(tc.tile_pool(name="sg", bufs=1))
            ps = ctx.enter_context(
                tc.tile_pool(name="ps", bufs=1, space="PSUM"))

            # ---- small input DMAs (gpsimd queue) ----
            wc = sg.tile([P, 3 * JH], f16)
            nc.gpsimd.dma_start(out=wc, in_=w_d)
            maskc = sg.tile([P, NCT], i32)
            nc.gpsimd.dma_start(out=maskc, in_=mask_d)
            bg_sb = sg.tile([1, 1], f32)
            nc.gpsimd.dma_start(out=bg_sb, in_=bg_d)
            bg_b = sg.tile([P, 1], f32)
            nc.gpsimd.partition_broadcast(bg_b, bg_sb)

            # ---- big input DMAs: even ctx chunks on the sync HWDGE
            # ring; hid + odd ctx chunks on the ACT HWDGE ring (the ACT
            # engine only pays ~0.6us issue residency per DMA, spaced
            # between gate batches). gpsimd/SWDGE moves no bulk data. ----
            def ctx_chunk_dma(k):
                c0, c1 = CB[k] * P, CB[k + 1] * P
                eng = nc.sync if k % 2 == 0 else nc.scalar
                eng.dma_start(out=ctx_sb[:, :, c0:c1],
                              in_=ctx_d[:, :, c0:c1])

            hid_sb = sg.tile([P, JH, G], f16)
            ctx_sb = sg.tile([P, JH, C], f16)
            nc.scalar.dma_start(out=hid_sb, in_=hid_d)
            ctx_chunk_dma(1)
            for k in (0, 2, 4, 6):
                ctx_chunk_dma(k)

            ones128 = sg.tile([P, 1], f32)
            nc.vector.memset(ones128, 1.0)
            dots = ps.tile([P, 2 * NCT], f32, tag="dots")
            gc_cols = sg.tile([P, NCT], f32)
            e_cols = sg.tile([P, NCT], f32)
            s1 = sg.tile([P, NCT], f32)
            s2 = sg.tile([P, NCT], f32)
            msc = sg.tile([P, NCT], f32)
            nc.vector.memset(msc, -30.0)
            out_sb = sg.tile([P, NCT, G], bf16)

            def chunk_dots(k):
                for ci in range(CB[k], CB[k + 1]):
                    for jh in range(JH):
                        nc.tensor.matmul(
                            dots[:, 2 * ci:2 * ci + 2],
                            ctx_sb[:, jh, ci * P:(ci + 1) * P],
                            wc[:, jh * 3:jh * 3 + 2],
                            start=(jh == 0), stop=(jh == JH - 1))

            def chunk_pre(k):
                t0, t1 = CB[k], CB[k + 1]
                # rhs col 0 = wg_c -> even dot cols are gc, odd are sc;
                # fold the gate bias b0 in here (per-partition add)
                nc.vector.tensor_scalar(
                    out=gc_cols[:, t0:t1], in0=dots[:, 2 * t0:2 * t1:2],
                    scalar1=bg_b[:, 0:1], scalar2=None, op0=ADD)
                for ci in range(t0, t1):
                    nc.scalar.activation(
                        out_sb[:, ci, :], ghb,
                        mybir.ActivationFunctionType.Sigmoid,
                        bias=gc_cols[:, ci:ci + 1])
                nc.vector.copy_predicated(
                    msc[:, t0:t1], maskc[:, t0:t1],
                    dots[:, 2 * t0 + 1:2 * t1:2])

            def pair_e(i):
                # e = exp(msc) = sig(msc)/sig(-msc) over chunk pair
                # (2i, 2i+1); same ACT table as the gates -> no reload
                a, b = CB[2 * i], CB[2 * i + 2]
                nc.scalar.activation(s1[:, a:b], msc[:, a:b],
                                     mybir.ActivationFunctionType.Sigmoid)
                nc.scalar.activation(s2[:, a:b], msc[:, a:b],
                                     mybir.ActivationFunctionType.Sigmoid,
                                     scale=-1.0)
                nc.vector.reciprocal(s2[:, a:b], s2[:, a:b])
                nc.vector.tensor_mul(e_cols[:, a:b], s1[:, a:b], s2[:, a:b])

            def chunk_ge(k):
                for ci in range(CB[k], CB[k + 1]):
                    nc.vector.tensor_scalar(
                        out=out_sb[:, ci, :], in0=out_sb[:, ci, :],
                        scalar1=e_cols[:, ci:ci + 1], scalar2=None,
                        op0=MULT)

            # ---- gh[g] = hid[g,:] @ wg_h -> broadcast to [128, G] ----
            # (PE does chunk-0 dots first: ctx0 and hid land in parallel)
            chunk_dots(0)
            ghp = ps.tile([1, G], f32, tag="ghp")
            for jh in range(JH):
                nc.tensor.matmul(ghp, wc[:, jh * 3 + 2:jh * 3 + 3],
                                 hid_sb[:, jh, :],
                                 start=(jh == 0), stop=(jh == JH - 1))
            gh_row = sg.tile([1, G], f32)
            nc.scalar.copy(gh_row, ghp)
            ghb = sg.tile([P, G], f32)
            nc.gpsimd.partition_broadcast(ghb, gh_row)

            chunk_pre(0)
            ctx_chunk_dma(3)
            chunk_dots(1)
            chunk_pre(1)
            pair_e(0)
            chunk_ge(0)
            chunk_ge(1)
            ctx_chunk_dma(5)
            for k in (2, 3):
                chunk_dots(k)
                chunk_pre(k)
            pair_e(1)
            chunk_ge(2)
            chunk_ge(3)
            ctx_chunk_dma(7)
            for k in (4, 5):
                chunk_dots(k)
                chunk_pre(k)
            pair_e(2)
            chunk_ge(4)
            chunk_ge(5)
            for k in (6, 7):
                chunk_dots(k)
                chunk_pre(k)
            pair_e(3)
            chunk_ge(6)
            chunk_ge(7)

            # ---- Z = sum_c e; rz = 1/Z broadcast ----
            zred = sg.tile([P, 1], f32)
            nc.vector.reduce_sum(zred, e_cols, axis=mybir.AxisListType.X)
            zp = ps.tile([1, 1], f32, tag="zp")
            nc.tensor.matmul(zp, zred, ones128, start=True, stop=True)
            rz = sg.tile([1, 1], f32)
            nc.vector.reciprocal(rz, zp)
            rz_b = sg.tile([P, 1], f32)
            nc.gpsimd.partition_broadcast(rz_b, rz)

            # ---- finals: one *rz per chunk (even on DVE, odd on ACT),
            # out-DMA stream split across sync + ACT HWDGE rings ----
            for k in range(NCH):
                t0, t1 = CB[k], CB[k + 1]
                if k % 2 == 0:
                    nc.vector.tensor_scalar(
                        out=out_sb[:, t0:t1, :], in0=out_sb[:, t0:t1, :],
                        scalar1=rz_b[:, 0:1], scalar2=None, op0=MULT)
                    nc.sync.dma_start(out=out_d[:, t0:t1, :],
                                      in_=out_sb[:, t0:t1, :])
                else:
                    nc.scalar.mul(out_sb[:, t0:t1, :], out_sb[:, t0:t1, :],
                                  rz_b[:, 0:1])
                    nc.scalar.dma_start(out=out_d[:, t0:t1, :],
                                        in_=out_sb[:, t0:t1, :])

    nc.compile()
    return nc


def _get_nc():
    if "nc" not in _cache:
        _cache["nc"] = _build()
    return _cache["nc"]


def make_in_maps(hidden_states, context_hidden, w_attn, w_gate, b_gate,
                 copy_mask):
    # wcols[p, jh*3+s]: s=0 wg_c, s=1 wa_c, s=2 wg_h at h=jh*128+p
    w3 = np.stack([w_gate[H:], w_attn[H:], w_gate[:H]], axis=1)  # [H, 3]
    wcols = np.ascontiguousarray(
        w3.reshape(JH, P, 3).transpose(1, 0, 2).reshape(P, 3 * JH)
    ).astype(np.float16)
    bg = np.asarray(b_gate, dtype=np.float32).reshape(1, 1)
    in_maps = []
    for b in range(B):
        ctxT = context_hidden[b].T.astype(np.float16)  # [H, C]
        ctxb = np.ascontiguousarray(
            ctxT.reshape(JH, P, C).transpose(1, 0, 2))
        hidT = hidden_states[b].T.astype(np.float16)  # [H, G]
        hidb = np.ascontiguousarray(
            hidT.reshape(JH, P, G).transpose(1, 0, 2))
        maskc = np.ascontiguousarray(
            copy_mask[b].reshape(NCT, P).T.astype(np.int32))
        in_maps.append({
            "ctx": ctxb, "hid": hidb, "w": wcols, "mask": maskc, "bg": bg,
        })
    return in_maps


def unpack_out(res):
    outs = []
    for b in range(B):
        outb = np.asarray(res.results[b]["out"])  # [P, NCT, G] bf16
        outs.append(
            outb.transpose(2, 1, 0).reshape(G, C).astype(np.float32))
    return np.stack(outs, axis=0)


def kernel(hidden_states, context_hidden, encoder_output, w_attn, w_gate,
           b_gate, copy_mask):
    from concourse.bass_utils import run_bass_kernel_spmd

    nc = _get_nc()
    in_maps = make_in_maps(hidden_states, context_hidden, w_attn, w_gate,
                           b_gate, copy_mask)
    res = run_bass_kernel_spmd(nc, in_maps, core_ids=list(range(N_CORES)))
    return unpack_out(res)


# revision 11
# speedup vs baseline: 2.0824x; 1.0045x over previous
"""Trainium2 Bass kernel for nn_CopyMechanism (optimized, v3).

Math (per batch b):
  out[g,c] = softmax_c(mask ? (score_h[g]+score_c[c]) : -inf)
             * sigmoid(gate_h[g]+gate_c[c]+b0)

softmax_c(score_h[g]+score_c[c]) == softmax_c(score_c): score_h drops out,
so copy_probs is independent of g and w_attn[:H] is unused; encoder_output
is unused by the reference. Scores are O(1): no max subtraction needed.

Layout strategy: everything is pre-blocked on the host so no on-chip
transposes are needed and all HBM traffic is fp16/bf16 (13MB/core):
  - ctx arrives as [h_p=128, jh=8, c=4096] fp16; PE matmuls with the ctx
    128x128 block *stationary* (fp16 -> FWL fast weight loads) and the
    (wg_c, wa_c) column pair *moving* put sc/gc on c-partitions directly.
  - softmax + all scalings are per-partition scalar ops; gate tiles are
    [c_p=128, g=512]: ACT sigmoid(ghb + gc bias), gh broadcast once.
  - out accumulates as bf16 [128, ci, g]; host unblocks to [G, C] f32.
Pipelining: ctx streams in decreasing-size c-chunks on two DMA queues
(hid on the second queue first); per chunk: PE dots -> DVE gc copy ->
ACT gates + Exp(masked sc) -> DVE gate*e (no Z dependency). Tail is only:
tiny Z reduce, one per-chunk *1/Z tensor_scalar, and the out-DMA stream.
ACT is the pacer in the window (32*0.71us gates); DMA in+out ~37us total.
"""
import sys

if "/opt/trn_rl_repo" not in sys.path:
    sys.path.insert(0, "/opt/trn_rl_repo")

import numpy as np
from contextlib import ExitStack

B, G, C, H = 8, 512, 4096, 1024
N_CORES = 8
P = 128
JH = H // P            # 8 h-blocks of 128
NCT = C // P           # 32 c-tiles of 128
# ctx chunk sizes in c-tiles; even chunks stream on the sync HWDGE queue,
# odd chunks (+hid) on the gpsimd SWDGE queue; the last chunk is one tile
# so the only unoverlapped gate work is minimal
SZ = [5, 4, 5, 4, 5, 3, 5, 1]
NCH = len(SZ)
CB = [0]
for s in SZ:
    CB.append(CB[-1] + s)
assert CB[-1] == NCT

_cache = {}


def _build():
    import concourse.bass as bass
    import concourse.tile as tile
    from concourse import bacc, mybir

    f32 = mybir.dt.float32
    f16 = mybir.dt.float16
    bf16 = mybir.dt.bfloat16
    i32 = mybir.dt.int32
    ADD = mybir.AluOpType.add
    MULT = mybir.AluOpType.mult

    nc = bacc.Bacc("TRN2", target_bir_lowering=False, debug=False,
                   num_devices=N_CORES)
    # ctxb[p, jh, c] = ctx[c, jh*128+p]  (fp16)
    ctx_d = nc.dram_tensor("ctx", [P, JH, C], f16, kind="ExternalInput").ap()
    # hidb[p, jh, g] = hid[g, jh*128+p]  (fp16)
    hid_d = nc.dram_tensor("hid", [P, JH, G], f16, kind="ExternalInput").ap()
    # wcols[p, jh*3+s]: s=0 wg_c, s=1 wa_c, s=2 wg_h  at h=jh*128+p  (fp16)
    w_d = nc.dram_tensor("w", [P, 3 * JH], f16, kind="ExternalInput").ap()
    # maskc[p, ci] = copy_mask[ci*128+p]
    mask_d = nc.dram_tensor("mask", [P, NCT], i32, kind="ExternalInput").ap()
    bg_d = nc.dram_tensor("bg", [1, 1], f32, kind="ExternalInput").ap()
    # outb[p, ci, g] = out[g, ci*128+p]  (bf16)
    out_d = nc.dram_tensor("out", [P, NCT, G], bf16,
                           kind="ExternalOutput").ap()

    with tile.TileContext(nc) as tc:
        with ExitStack() as ctx:
            sg = ctx.enter_context(tc.tile_pool(name="sg", bufs=1))
            ps = ctx.enter_context(
                tc.tile_pool(name="ps", bufs=1, space="PSUM"))

            # ---- small input DMAs (gpsimd queue) ----
            wc = sg.tile([P, 3 * JH], f16)
            nc.gpsimd.dma_start(out=wc, in_=w_d)
            maskc = sg.tile([P, NCT], i32)
            nc.gpsimd.dma_start(out=maskc, in_=mask_d)
            bg_sb = sg.tile([1, 1], f32)
            nc.gpsimd.dma_start(out=bg_sb, in_=bg_d)
            bg_b = sg.tile([P, 1], f32)
            nc.gpsimd.partition_broadcast(bg_b, bg_sb)

            # ---- big input DMAs: even ctx chunks on the sync HWDGE
            # ring; hid + odd ctx chunks on the ACT HWDGE ring (the ACT
            # engine only pays ~0.6us issue residency per DMA, spaced
            # between gate batches). gpsimd/SWDGE moves no bulk data. ----
            def ctx_chunk_dma(k):
                c0, c1 = CB[k] * P, CB[k + 1] * P
                nc.sync.dma_start(out=ctx_sb[:, :, c0:c1],
                                  in_=ctx_d[:, :, c0:c1])

            hid_sb = sg.tile([P, JH, G], f16)
            ctx_sb = sg.tile([P, JH, C], f16)
            nc.scalar.dma_start(out=hid_sb, in_=hid_d)
            for k in range(NCH):
                ctx_chunk_dma(k)

            ones128 = sg.tile([P, 1], f32)
            nc.vector.memset(ones128, 1.0)
            dots = ps.tile([P, 2 * NCT], f32, tag="dots")
            gc_cols = sg.tile([P, NCT], f32)
            e_cols = sg.tile([P, NCT], f32)
            s1 = sg.tile([P, NCT], f32)
            s2 = sg.tile([P, NCT], f32)
            msc = sg.tile([P, NCT], f32)
            nc.vector.memset(msc, -30.0)
            out_sb = sg.tile([P, NCT, G], bf16)

            def chunk_dots(k):
                for ci in range(CB[k], CB[k + 1]):
                    for jh in range(JH):
                        nc.tensor.matmul(
                            dots[:, 2 * ci:2 * ci + 2],
                            ctx_sb[:, jh, ci * P:(ci + 1) * P],
                            wc[:, jh * 3:jh * 3 + 2],
                            start=(jh == 0), stop=(jh == JH - 1))

            def chunk_pre(k):
                t0, t1 = CB[k], CB[k + 1]
                # rhs col 0 = wg_c -> even dot cols are gc, odd are sc;
                # fold the gate bias b0 in here (per-partition add)
                nc.vector.tensor_scalar(
                    out=gc_cols[:, t0:t1], in0=dots[:, 2 * t0:2 * t1:2],
                    scalar1=bg_b[:, 0:1], scalar2=None, op0=ADD)
                for ci in range(t0, t1):
                    nc.scalar.activation(
                        out_sb[:, ci, :], ghb,
                        mybir.ActivationFunctionType.Sigmoid,
                        bias=gc_cols[:, ci:ci + 1])
                nc.vector.copy_predicated(
                    msc[:, t0:t1], maskc[:, t0:t1],
                    dots[:, 2 * t0 + 1:2 * t1:2])

            def pair_e(i):
                # e = exp(msc) = sig(msc)/sig(-msc) over chunk pair
                # (2i, 2i+1); same ACT table as the gates -> no reload
                a, b = CB[2 * i], CB[2 * i + 2]
                nc.scalar.activation(s1[:, a:b], msc[:, a:b],
                                     mybir.ActivationFunctionType.Sigmoid)
                nc.scalar.activation(s2[:, a:b], msc[:, a:b],
                                     mybir.ActivationFunctionType.Sigmoid,
                                     scale=-1.0)
                nc.vector.reciprocal(s2[:, a:b], s2[:, a:b])
                nc.vector.tensor_mul(e_cols[:, a:b], s1[:, a:b], s2[:, a:b])

            def chunk_ge(k):
                for ci in range(CB[k], CB[k + 1]):
                    nc.vector.tensor_scalar(
                        out=out_sb[:, ci, :], in0=out_sb[:, ci, :],
                        scalar1=e_cols[:, ci:ci + 1], scalar2=None,
                        op0=MULT)

            # ---- gh[g] = hid[g,:] @ wg_h -> broadcast to [128, G] ----
            # (hid lands first on the ACT ring, so PE does gh first)
            ghp = ps.tile([1, G], f32, tag="ghp")
            for jh in range(JH):
                nc.tensor.matmul(ghp, wc[:, jh * 3 + 2:jh * 3 + 3],
                                 hid_sb[:, jh, :],
                                 start=(jh == 0), stop=(jh == JH - 1))
            gh_row = sg.tile([1, G], f32)
            nc.scalar.copy(gh_row, ghp)
            ghb = sg.tile([P, G], f32)
            nc.gpsimd.partition_broadcast(ghb, gh_row)

            for i in range(NCH // 2):
                for k in (2 * i, 2 * i + 1):
                    chunk_dots(k)
                    chunk_pre(k)
                pair_e(i)
                chunk_ge(2 * i)
                chunk_ge(2 * i + 1)

            # ---- Z = sum_c e; rz = 1/Z broadcast ----
            zred = sg.tile([P, 1], f32)
            nc.vector.reduce_sum(zred, e_cols, axis=mybir.AxisListType.X)
            zp = ps.tile([1, 1], f32, tag="zp")
            nc.tensor.matmul(zp, zred, ones128, start=True, stop=True)
            rz = sg.tile([1, 1], f32)
            nc.vector.reciprocal(rz, zp)
            rz_b = sg.tile([P, 1], f32)
            nc.gpsimd.partition_broadcast(rz_b, rz)

            # ---- finals: one *rz per chunk (even on DVE, odd on ACT),
            # out-DMA stream split across sync + ACT HWDGE rings ----
            for k in range(NCH):
                t0, t1 = CB[k], CB[k + 1]
                if k % 2 == 0:
                    nc.vector.tensor_scalar(
                        out=out_sb[:, t0:t1, :], in0=out_sb[:, t0:t1, :],
                        scalar1=rz_b[:, 0:1], scalar2=None, op0=MULT)
                    nc.sync.dma_start(out=out_d[:, t0:t1, :],
                                      in_=out_sb[:, t0:t1, :])
                else:
                    nc.scalar.mul(out_sb[:, t0:t1, :], out_sb[:, t0:t1, :],
                                  rz_b[:, 0:1])
                    nc.scalar.dma_start(out=out_d[:, t0:t1, :],
                                        in_=out_sb[:, t0:t1, :])

    nc.compile()
    return nc


def _get_nc():
    if "nc" not in _cache:
        _cache["nc"] = _build()
    return _cache["nc"]


def make_in_maps(hidden_states, context_hidden, w_attn, w_gate, b_gate,
                 copy_mask):
    # wcols[p, jh*3+s]: s=0 wg_c, s=1 wa_c, s=2 wg_h at h=jh*128+p
    w3 = np.stack([w_gate[H:], w_attn[H:], w_gate[:H]], axis=1)  # [H, 3]
    wcols = np.ascontiguousarray(
        w3.reshape(JH, P, 3).transpose(1, 0, 2).reshape(P, 3 * JH)
    ).astype(np.float16)
    bg = np.asarray(b_gate, dtype=np.float32).reshape(1, 1)
    in_maps = []
    for b in range(B):
        ctxT = context_hidden[b].T.astype(np.float16)  # [H, C]
        ctxb = np.ascontiguousarray(
            ctxT.reshape(JH, P, C).transpose(1, 0, 2))
        hidT = hidden_states[b].T.astype(np.float16)  # [H, G]
        hidb = np.ascontiguousarray(
            hidT.reshape(JH, P, G).transpose(1, 0, 2))
        maskc = np.ascontiguousarray(
            copy_mask[b].reshape(NCT, P).T.astype(np.int32))
        in_maps.append({
            "ctx": ctxb, "hid": hidb, "w": wcols, "mask": maskc, "bg": bg,
        })
    return in_maps


def unpack_out(res):
    outs = []
    for b in range(B):
        outb = np.asarray(res.results[b]["out"])  # [P, NCT, G] bf16
        outs.append(
            outb.transpose(2, 1, 0).reshape(G, C).astype(np.float32))
    return np.stack(outs, axis=0)


def kernel(hidden_states, context_hidden, encoder_output, w_attn, w_gate,
           b_gate, copy_mask):
    from concourse.bass_utils import run_bass_kernel_spmd

    nc = _get_nc()
    in_maps = make_in_maps(hidden_states, context_hidden, w_attn, w_gate,
                           b_gate, copy_mask)
    res = run_bass_kernel_spmd(nc, in_maps, core_ids=list(range(N_CORES)))
    return unpack_out(res)


# revision 19
# speedup vs baseline: 2.3209x; 1.1145x over previous
"""Trainium2 Bass kernel for nn_CopyMechanism (optimized, v3).

Math (per batch b):
  out[g,c] = softmax_c(mask ? (score_h[g]+score_c[c]) : -inf)
             * sigmoid(gate_h[g]+gate_c[c]+b0)

softmax_c(score_h[g]+score_c[c]) == softmax_c(score_c): score_h drops out,
so copy_probs is independent of g and w_attn[:H] is unused; encoder_output
is unused by the reference. Scores are O(1): no max subtraction needed.

Layout strategy: everything is pre-blocked on the host so no on-chip
transposes are needed and all HBM traffic is fp16/bf16 (13MB/core):
  - ctx arrives as [h_p=128, jh=8, c=4096] fp16; PE matmuls with the ctx
    128x128 block *stationary* (fp16 -> FWL fast weight loads) and the
    (wg_c, wa_c) column pair *moving* put sc/gc on c-partitions directly.
  - softmax + all scalings are per-partition scalar ops; gate tiles are
    [c_p=128, g=512]: ACT sigmoid(ghb + gc bias), gh broadcast once.
  - out accumulates as bf16 [128, ci, g]; host unblocks to [G, C] f32.
Pipelining: ctx streams in decreasing-size c-chunks on two DMA queues
(hid on the second queue first); per chunk: PE dots -> DVE gc copy ->
ACT gates + Exp(masked sc) -> DVE gate*e (no Z dependency). Tail is only:
tiny Z reduce, one per-chunk *1/Z tensor_scalar, and the out-DMA stream.
ACT is the pacer in the window (32*0.71us gates); DMA in+out ~37us total.
"""
import sys

if "/opt/trn_rl_repo" not in sys.path:
    sys.path.insert(0, "/opt/trn_rl_repo")

import numpy as np
from contextlib import ExitStack

B, G, C, H = 8, 512, 4096, 1024
N_CORES = 8
P = 128
JH = H // P            # 8 h-blocks of 128
NCT = C // P           # 32 c-tiles of 128
# ctx chunk sizes in c-tiles; even chunks stream on the sync HWDGE queue,
# odd chunks (+hid) on the gpsimd SWDGE queue; the last chunk is one tile
# so the only unoverlapped gate work is minimal
SZ = [2, 4, 5, 5, 5, 5, 5, 1]
NCH = len(SZ)
CB = [0]
for s in SZ:
    CB.append(CB[-1] + s)
assert CB[-1] == NCT

_cache = {}


def _build():
    import concourse.bass as bass
    import concourse.tile as tile
    from concourse import bacc, mybir

    f32 = mybir.dt.float32
    f16 = mybir.dt.float16
    bf16 = mybir.dt.bfloat16
    i32 = mybir.dt.int32
    ADD = mybir.AluOpType.add
    MULT = mybir.AluOpType.mult

    nc = bacc.Bacc("TRN2", target_bir_lowering=False, debug=False,
                   num_devices=N_CORES)
    # ctxb[p, jh, c] = ctx[c, jh*128+p]  (fp16)
    ctx_d = nc.dram_tensor("ctx", [P, JH, C], f16, kind="ExternalInput").ap()
    # hidb[p, jh, g] = hid[g, jh*128+p]  (fp16)
    hid_d = nc.dram_tensor("hid", [P, JH, G], f16, kind="ExternalInput").ap()
    # wcols[p, jh*3+s]: s=0 wg_c, s=1 wa_c, s=2 wg_h  at h=jh*128+p  (fp16)
    w_d = nc.dram_tensor("w", [P, 3 * JH], f16, kind="ExternalInput").ap()
    # maskc[p, ci] = copy_mask[ci*128+p]
    mask_d = nc.dram_tensor("mask", [P, NCT], i32, kind="ExternalInput").ap()
    bg_d = nc.dram_tensor("bg", [1, 1], f32, kind="ExternalInput").ap()
    # outb[p, ci, g] = out[g, ci*128+p]  (bf16)
    out_d = nc.dram_tensor("out", [P, NCT, G], bf16,
                           kind="ExternalOutput").ap()

    with tile.TileContext(nc) as tc:
        with ExitStack() as ctx:
            sg = ctx.enter_context(tc.tile_pool(name="sg", bufs=1))
            ps = ctx.enter_context(
                tc.tile_pool(name="ps", bufs=1, space="PSUM"))

            # ---- small input DMAs (gpsimd queue) ----
            wc = sg.tile([P, 3 * JH], f16)
            nc.gpsimd.dma_start(out=wc, in_=w_d)
            maskc = sg.tile([P, NCT], i32)
            nc.gpsimd.dma_start(out=maskc, in_=mask_d)
            # bg broadcast straight from DRAM via stride-0 DMA (gpsimd
            # partition_broadcast has a huge cold-start; avoid it)
            bg_b = sg.tile([P, 1], f32)
            nc.gpsimd.dma_start(
                out=bg_b,
                in_=bass.AP(tensor=bg_d.tensor, offset=bg_d.offset,
                            ap=[[0, P], [1, 1]]))

            # dummy sigmoid so the ACT table loads during the lead-in
            dummy = sg.tile([1, 1], f32)
            nc.vector.memset(dummy, 0.0)
            nc.scalar.activation(dummy, dummy,
                                 mybir.ActivationFunctionType.Sigmoid)

            # ---- big input DMAs: even ctx chunks on the sync HWDGE
            # ring; hid + odd ctx chunks on the ACT HWDGE ring (the ACT
            # engine only pays ~0.6us issue residency per DMA, spaced
            # between gate batches). gpsimd/SWDGE moves no bulk data. ----
            def ctx_chunk_dma(k):
                c0, c1 = CB[k] * P, CB[k + 1] * P
                nc.sync.dma_start(out=ctx_sb[:, :, c0:c1],
                                  in_=ctx_d[:, :, c0:c1])

            hid_sb = sg.tile([P, JH, G], f16)
            ctx_sb = sg.tile([P, JH, C], f16)
            nc.scalar.dma_start(out=hid_sb, in_=hid_d)
            for k in range(NCH):
                ctx_chunk_dma(k)

            ones128 = sg.tile([P, 1], f32)
            nc.vector.memset(ones128, 1.0)
            ones_row = sg.tile([1, P], f32)
            nc.vector.memset(ones_row, 1.0)
            dots = ps.tile([P, 2 * NCT], f32, tag="dots")
            gc_cols = sg.tile([P, NCT], f32)
            e_cols = sg.tile([P, NCT], f32)
            s1 = sg.tile([P, NCT], f32)
            s2 = sg.tile([P, NCT], f32)
            msc = sg.tile([P, NCT], f32)
            nc.vector.memset(msc, -30.0)
            out_sb = sg.tile([P, NCT, G], bf16)

            def chunk_dots(k):
                for ci in range(CB[k], CB[k + 1]):
                    for jh in range(JH):
                        nc.tensor.matmul(
                            dots[:, 2 * ci:2 * ci + 2],
                            ctx_sb[:, jh, ci * P:(ci + 1) * P],
                            wc[:, jh * 3:jh * 3 + 2],
                            start=(jh == 0), stop=(jh == JH - 1))

            def chunk_pre(k):
                t0, t1 = CB[k], CB[k + 1]
                # rhs col 0 = wg_c -> even dot cols are gc, odd are sc;
                # fold the gate bias b0 in here (per-partition add)
                nc.vector.tensor_scalar(
                    out=gc_cols[:, t0:t1], in0=dots[:, 2 * t0:2 * t1:2],
                    scalar1=bg_b[:, 0:1], scalar2=None, op0=ADD)
                for ci in range(t0, t1):
                    nc.scalar.activation(
                        out_sb[:, ci, :], ghb,
                        mybir.ActivationFunctionType.Sigmoid,
                        bias=gc_cols[:, ci:ci + 1])
                nc.vector.copy_predicated(
                    msc[:, t0:t1], maskc[:, t0:t1],
                    dots[:, 2 * t0 + 1:2 * t1:2])

            def pair_e(i):
                # e = exp(msc) = sig(msc)/sig(-msc) over chunk pair
                # (2i, 2i+1); same ACT table as the gates -> no reload
                a, b = CB[2 * i], CB[2 * i + 2]
                nc.scalar.activation(s1[:, a:b], msc[:, a:b],
                                     mybir.ActivationFunctionType.Sigmoid)
                nc.scalar.activation(s2[:, a:b], msc[:, a:b],
                                     mybir.ActivationFunctionType.Sigmoid,
                                     scale=-1.0)
                nc.vector.reciprocal(s2[:, a:b], s2[:, a:b])
                nc.vector.tensor_mul(e_cols[:, a:b], s1[:, a:b], s2[:, a:b])

            def chunk_ge(k):
                for ci in range(CB[k], CB[k + 1]):
                    nc.vector.tensor_scalar(
                        out=out_sb[:, ci, :], in0=out_sb[:, ci, :],
                        scalar1=e_cols[:, ci:ci + 1], scalar2=None,
                        op0=MULT)

            # ---- gh[g] = hid[g,:] @ wg_h -> broadcast to [128, G] ----
            # (hid lands first on the ACT ring, so PE does gh first)
            ghb = ps.tile([P, G], f32, tag="ghb")
            ghp = ps.tile([1, G], f32, tag="ghp")
            for jh in range(JH):
                nc.tensor.matmul(ghp, wc[:, jh * 3 + 2:jh * 3 + 3],
                                 hid_sb[:, jh, :],
                                 start=(jh == 0), stop=(jh == JH - 1))
            gh_row = sg.tile([1, G], f32)
            nc.scalar.copy(gh_row, ghp)

            # broadcast gh to all partitions with a K=1 PE matmul; the
            # gate activations read it straight from PSUM
            chunk_dots(0)
            nc.tensor.matmul(ghb, ones_row, gh_row, start=True, stop=True)

            chunk_pre(0)
            chunk_dots(1)
            chunk_pre(1)
            pair_e(0)
            chunk_ge(0)
            chunk_ge(1)
            for i in range(1, NCH // 2):
                for k in (2 * i, 2 * i + 1):
                    chunk_dots(k)
                    chunk_pre(k)
                pair_e(i)
                chunk_ge(2 * i)
                chunk_ge(2 * i + 1)

            # ---- Z = sum_c e; rz = 1/Z broadcast ----
            zred = sg.tile([P, 1], f32)
            nc.vector.reduce_sum(zred, e_cols, axis=mybir.AxisListType.X)
            zp = ps.tile([1, 1], f32, tag="zp")
            nc.tensor.matmul(zp, zred, ones128, start=True, stop=True)
            rz = sg.tile([1, 1], f32)
            nc.vector.reciprocal(rz, zp)
            # broadcast 1/Z to all partitions with a tiny PE matmul
            # (ones[1,128]^T @ rz), then stage to SBUF on DVE
            rzb_ps = ps.tile([P, 1], f32, tag="rzb")
            nc.tensor.matmul(rzb_ps, ones_row, rz, start=True, stop=True)
            rz_b = sg.tile([P, 1], f32)
            nc.vector.tensor_scalar(out=rz_b, in0=rzb_ps, scalar1=0.0,
                                    scalar2=None, op0=ADD)

            # ---- finals: one *rz per chunk (even on DVE, odd on ACT),
            # out-DMA stream split across sync + ACT HWDGE rings ----
            for k in range(NCH):
                t0, t1 = CB[k], CB[k + 1]
                if k % 2 == 0:
                    nc.vector.tensor_scalar(
                        out=out_sb[:, t0:t1, :], in0=out_sb[:, t0:t1, :],
                        scalar1=rz_b[:, 0:1], scalar2=None, op0=MULT)
                    nc.sync.dma_start(out=out_d[:, t0:t1, :],
                                      in_=out_sb[:, t0:t1, :])
                else:
                    nc.scalar.mul(out_sb[:, t0:t1, :], out_sb[:, t0:t1, :],
                                  rz_b[:, 0:1])
                    nc.scalar.dma_start(out=out_d[:, t0:t1, :],
                                        in_=out_sb[:, t0:t1, :])

    nc.compile()
    return nc


def _get_nc():
    if "nc" not in _cache:
        _cache["nc"] = _build()
    return _cache["nc"]


def make_in_maps(hidden_states, context_hidden, w_attn, w_gate, b_gate,
                 copy_mask):
    # wcols[p, jh*3+s]: s=0 wg_c, s=1 wa_c, s=2 wg_h at h=jh*128+p
    w3 = np.stack([w_gate[H:], w_attn[H:], w_gate[:H]], axis=1)  # [H, 3]
    wcols = np.ascontiguousarray(
        w3.reshape(JH, P, 3).transpose(1, 0, 2).reshape(P, 3 * JH)
    ).astype(np.float16)
    bg = np.asarray(b_gate, dtype=np.float32).reshape(1, 1)
    in_maps = []
    for b in range(B):
        ctxT = context_hidden[b].T.astype(np.float16)  # [H, C]
        ctxb = np.ascontiguousarray(
            ctxT.reshape(JH, P, C).transpose(1, 0, 2))
        hidT = hidden_states[b].T.astype(np.float16)  # [H, G]
        hidb = np.ascontiguousarray(
            hidT.reshape(JH, P, G).transpose(1, 0, 2))
        maskc = np.ascontiguousarray(
            copy_mask[b].reshape(NCT, P).T.astype(np.int32))
        in_maps.append({
            "ctx": ctxb, "hid": hidb, "w": wcols, "mask": maskc, "bg": bg,
        })
    return in_maps


def unpack_out(res):
    outs = []
    for b in range(B):
        outb = np.asarray(res.results[b]["out"])  # [P, NCT, G] bf16
        outs.append(
            outb.transpose(2, 1, 0).reshape(G, C).astype(np.float32))
    return np.stack(outs, axis=0)


def kernel(hidden_states, context_hidden, encoder_output, w_attn, w_gate,
           b_gate, copy_mask):
    from concourse.bass_utils import run_bass_kernel_spmd

    nc = _get_nc()
    in_maps = make_in_maps(hidden_states, context_hidden, w_attn, w_gate,
                           b_gate, copy_mask)
    res = run_bass_kernel_spmd(nc, in_maps, core_ids=list(range(N_CORES)))
    return unpack_out(res)
